# revision 30
# baseline (speedup 1.0000x reference)
"""Cross-attention kernel for Trainium2, 8 NeuronCores.

Sharding: core c handles batch b=c//2 and query-half th=c%2 (1024 of 2048
query rows), all 16 heads, full T_E=1024 keys. No cross-core reduction is
needed: each core produces complete [1024, 1024] slices of both outputs.

Host-side prep (free w.r.t. the device timeline): x-half and enc are
transposed to feature-major and laid out so every DMA is contiguous
per-partition; weights are pre-arranged to SBUF order; outputs are fp16 on
device and upcast on host.

Per-core pipeline:
  P1: qT = Wq.T @ xT (fp16 [co, t]); evac via ACT w/ bias.
  P2 (per head-pair ct, software-pipelined): k-proj for the pair's 128
      columns; v-proj in N=512 matmuls (two 8-head halves x four te-pair
      subtiles, interleaved into the qk stream; 128 matmuls vs 512);
      qk per te with the two K=64 head groups packed via tile_position;
      exp on ACT with bias -ln16 -> attx = exp(s)/16 fp16 (folds the
      head-mean /16; ones col of v_aug is 16 so the av sum row is the
      true rowsum, and Wp is pre-scaled x16 host-side); av matmul (M=65)
      interleaved per-te with the NEXT pair's qk so the PE never starves;
      reciprocal (fp16, DVE) -> gpsimd partition-broadcast R; y-norm
      (DVE); att_mean acc += attx * R (DVE 0:640 + gpsimd 640:1024; the
      final head is chunked by t-slice so the P3 transposes start early).
  P3: y = yT.T @ Wp (+bp via ones row) with fp16 out; then att_mean
      transposed back on PE (plain fp16 copies; mean already folded).

mask is all-False per the input spec (fill=zeros); if a nonzero mask is ever
passed, a numpy fallback computes the exact reference instead.
"""

import sys

sys.path.insert(0, "/opt/trn_rl_repo")

import numpy as np

import concourse.bass as bass
import concourse.tile as tile
from concourse import bacc, mybir
from concourse.bass_utils import run_bass_kernel_spmd
from concourse.masks import make_identity

F32 = mybir.dt.float32
F16 = mybir.dt.float16

T = 1024   # local query rows per core
TE = 1024  # encoder tokens
C = 1024   # embed dim
H = 16     # heads
DH = 64    # head dim
SCALE = 0.125  # 1/sqrt(64)

_NC = {}


def _build(vp_bias=True):
    nc = bacc.Bacc("TRN2", target_bir_lowering=False, debug=False, num_devices=8)
    xh = nc.declare_dram_parameter("xh", [128, 8, T], F16, isOutput=False)
    enc = nc.declare_dram_parameter("enc", [128, 8, TE], F16, isOutput=False)
    wq_d = nc.declare_dram_parameter("wq", [8, 128, 8, 128], F16, isOutput=False)
    wk_d = nc.declare_dram_parameter("wk", [8, 128, 8, 128], F16, isOutput=False)
    wv_d = nc.declare_dram_parameter("wv", [128, 8, C], F16, isOutput=False)
    wp_d = nc.declare_dram_parameter("wp", [128, 8, C], F16, isOutput=False)
    bq_d = nc.declare_dram_parameter("bq", [128, 8], F32, isOutput=False)
    bk_d = nc.declare_dram_parameter("bk", [128, 8], F32, isOutput=False)
    bv_d = nc.declare_dram_parameter("bv", [1, C], F16, isOutput=False)
    bp_d = nc.declare_dram_parameter("bp", [1, C], F16, isOutput=False)
    y_d = nc.declare_dram_parameter("y", [T, C], F16, isOutput=True)
    am_d = nc.declare_dram_parameter("am", [T, TE], F16, isOutput=True)

    Exp = mybir.ActivationFunctionType.Exp
    Ident = mybir.ActivationFunctionType.Identity

    with tile.TileContext(nc) as tc:
      with tc.tile_pool(name="persist", bufs=1) as persist:
        ident16 = persist.tile([128, 128], F16)
        make_identity(nc, ident16)

        qT = persist.tile([128, 8, T], F16)       # [co%128, co//128, t]
        v_aug = persist.tile([128, 8, H, 66], F16)  # [te%128, te//128, h, .]
        yT = persist.tile([128, 8, T], F16)       # [c%128, c//8, t]
        am_a = persist.tile([128, 8, T], F16)     # [te%128, te//128, t]
        bq_sb = persist.tile([128, 8], F32)
        bk_sb = persist.tile([128, 8], F32)
        if vp_bias:
            ones_t = persist.tile([1, T], F16)    # proj bias row (lhsT)
            bv_row = persist.tile([1, C], F16)    # rhs bias row for v
            bp_row = persist.tile([1, C], F16)    # rhs bias row for proj
            ones128 = persist.tile([1, 128], F16)  # lhsT ones row for v bias

        # head-mean fold: attx is stored as exp(s)/16 (exp bias -ln16), the
        # ones column is 16 so the av sum row is the true rowsum, and Wp is
        # pre-scaled x16 host-side to undo the /16 on y.
        nln16 = persist.tile([128, 1], F32)
        nc.vector.memset(nln16, -2.772588722239781)
        nc.vector.memset(v_aug[:, :, :, 64:66], 0.0)
        nc.vector.memset(v_aug[:, :, :, 64:65], 16.0)
        if vp_bias:
            nc.vector.memset(ones_t, 1.0)
            nc.vector.memset(ones128, 1.0)

        with tc.tile_pool(name="mid", bufs=1) as mid, \
             tc.tile_pool(name="psA", bufs=2, space="PSUM") as psA:
            # ---- long-lived loads (enc/wv/wp survive into P2/P3) ----
            encT = mid.tile([128, 8, TE], F16)
            wv_sb = mid.tile([128, 8, C], F16)
            wp_sb = mid.tile([128, 8, C], F16)

            with tc.tile_pool(name="early", bufs=1) as early:
                xT = early.tile([128, 8, T], F16)
                wq_sb = early.tile([128, 8, C], F16)
                # priority order: q-proj inputs first, split for earliest
                # PE start
                wq_cs = [wq_sb[:, :, co * 128:(co + 1) * 128]
                         for co in range(8)]
                nc.sync.dma_start(out=xT[:, :, 0:512], in_=xh[:, :, 0:512])
                nc.sync.dma_start(out=wq_cs[0], in_=wq_d[0])
                nc.sync.dma_start(out=xT[:, :, 512:T], in_=xh[:, :, 512:T])
                for co in range(1, 8):
                    nc.sync.dma_start(out=wq_cs[co], in_=wq_d[co])
                nc.sync.dma_start(out=bq_sb, in_=bq_d[:, :])
                nc.sync.dma_start(out=bk_sb, in_=bk_d[:, :])
                nc.sync.dma_start(out=encT, in_=enc[:, :, :])
                nc.sync.dma_start(out=wv_sb, in_=wv_d[:, :, :])
                if vp_bias:
                    nc.sync.dma_start(out=bv_row, in_=bv_d[:, :])
                    nc.sync.dma_start(out=bp_row, in_=bp_d[:, :])
                # wp is loaded later (inside the ct loop) so the early wk
                # chunk DMAs are not queued behind this 2MB transfer

                # ---------------- P1: q projection ----------------
                for co in range(8):
                    psq = psA.tile([128, 1024], F32, tag="pp", bufs=2)
                    for t2 in range(2):
                        for ci in range(8):
                            nc.tensor.matmul(
                                psq[:, t2 * 512:(t2 + 1) * 512],
                                wq_sb[:, ci, co * 128:(co + 1) * 128],
                                xT[:, ci, t2 * 512:(t2 + 1) * 512],
                                start=(ci == 0),
                                stop=(ci == 7),
                            )
                    nc.scalar.activation(
                        out=qT[:, co, :], in_=psq, func=Ident,
                        bias=bq_sb[:, co:co + 1], scale=1.0,
                    )

            # ---------------- P2: k/v-proj + attention, pipelined --------
            with tc.tile_pool(name="p2", bufs=1) as p2:
                state = {}  # per-ct live tiles: attxs, psys, kchunk

                def k_proj(ct):
                    # k projection for this pair's 128 co-columns
                    wk_ct = p2.tile([128, 8, 128], F16, tag="wk", bufs=2,
                                    name=f"wk{ct}")
                    nc.sync.dma_start(out=wk_ct, in_=wk_d[ct, :, :, :])
                    kc = p2.tile([128, TE], F16, tag="kc", bufs=2,
                                 name=f"kc{ct}")
                    psk = psA.tile([128, 1024], F32, tag="pp", bufs=2,
                                   name=f"psk{ct}")
                    for t2 in range(2):
                        for ci in range(8):
                            nc.tensor.matmul(
                                psk[:, t2 * 512:(t2 + 1) * 512],
                                wk_ct[:, ci, :],
                                encT[:, ci, t2 * 512:(t2 + 1) * 512],
                                start=(ci == 0),
                                stop=(ci == 7),
                            )
                    nc.scalar.activation(
                        out=kc, in_=psk, func=Ident,
                        bias=bk_sb[:, ct:ct + 1], scale=1.0,
                    )
                    return kc

                def v_sub(half, tp):
                    # v projection for co-half `half` (8 heads), te chunks
                    # 2*tp and 2*tp+1, with full-width N=512 matmuls (one
                    # accumulation group per 512-col psum bank)
                    psv = psA.tile([128, 2, 8, 64], F32, tag="pp", bufs=2,
                                   name=f"psv{half}_{tp}")
                    for j in range(2):
                        te = 2 * tp + j
                        for ci in range(8):
                            nc.tensor.matmul(
                                psv[:, j, :, :],
                                encT[:, ci, te * 128:(te + 1) * 128],
                                wv_sb[:, ci, half * 512:(half + 1) * 512],
                                start=(ci == 0),
                                stop=(ci == 7 and not vp_bias),
                            )
                        if vp_bias:
                            nc.tensor.matmul(
                                psv[:, j, :, :],
                                ones128,
                                bv_row[:, half * 512:(half + 1) * 512],
                                start=False, stop=True,
                            )
                    nc.scalar.copy(
                        v_aug[:, 2 * tp:2 * tp + 2,
                              8 * half:8 * half + 8, 0:64], psv)

                def qk_alloc(ct, kc):
                    attxs = [p2.tile([128, 8, T], F16, tag="attx", bufs=4,
                                     name=f"attx{ct}g{g}") for g in range(2)]
                    psys = [psA.tile([65, 1024], F32, tag="py", bufs=2,
                                     name=f"psy{ct}g{g}") for g in range(2)]
                    state[ct] = (attxs, psys, kc)

                def qk_te(ct, te):
                    attxs, _, kc = state[ct]
                    psas = [psA.tile([128, 1024], F32, tag="pp", bufs=2,
                                     name=f"psa{ct}t{te}g{g}") for g in range(2)]
                    for t2 in range(2):
                        for g, hp in enumerate((0, 64)):
                            nc.tensor.matmul(
                                psas[g][:, t2 * 512:(t2 + 1) * 512],
                                kc[hp:hp + 64, te * 128:(te + 1) * 128],
                                qT[hp:hp + 64, ct, t2 * 512:(t2 + 1) * 512],
                                start=True, stop=True,
                                tile_position=(hp, 0),
                            )
                    for g in range(2):
                        nc.scalar.activation(
                            out=attxs[g][:, te, :], in_=psas[g],
                            func=Exp, scale=SCALE, bias=nln16[:, 0:1],
                        )

                def av_te(ct, te):
                    attxs, psys, _ = state[ct]
                    for g, hp in enumerate((0, 64)):
                        h = 2 * ct + g
                        for t2 in range(2):
                            nc.tensor.matmul(
                                psys[g][:, t2 * 512:(t2 + 1) * 512],
                                v_aug[:, te, h, 0:65],
                                attxs[g][:, te, t2 * 512:(t2 + 1) * 512],
                                start=(te == 0), stop=(te == 7),
                            )

                def tail(ct):
                    attxs, psys, _ = state.pop(ct)
                    Rs = []
                    # recips + broadcasts + ynorms first: frees the psy psum
                    # slots early so the next pair's av is not gated on the
                    # att_mean accumulation below
                    for g in range(2):
                        r16 = p2.tile([1, T], F16, tag="r16", bufs=2,
                                      name=f"r{ct}g{g}")
                        with nc.allow_low_precision("softmax recip fp16"):
                            nc.vector.reciprocal(r16, psys[g][64:65, :])
                        R = p2.tile([128, T], F16, tag="R", bufs=2,
                                    name=f"R{ct}g{g}")
                        nc.gpsimd.partition_broadcast(R, r16)
                        Rs.append(R)
                    for g, hp in enumerate((0, 64)):
                        nc.vector.tensor_mul(
                            yT[hp:hp + 64, ct, :], psys[g][0:64, :],
                            Rs[g][0:64, :]
                        )
                    # att_mean accumulator += attx * R (normalize attx
                    # in place, then accumulate; half of the g1 add runs on
                    # the mostly-idle Pool engine to balance DVE).  The final
                    # head (ct=7 g=1) is chunked by t so the P3 am transposes
                    # can start per-slice instead of waiting for one big add.
                    for g in range(2):
                        attx, R = attxs[g], Rs[g]
                        R_b = bass.AP(
                            tensor=R.tensor, offset=R.offset,
                            ap=[R.ap[0], [0, 8], R.ap[1]],
                        )
                        if ct == 7 and g == 1:
                            for tt in range(8):
                                sl = slice(tt * 128, (tt + 1) * 128)
                                Rsl = R[:, sl]
                                R_bsl = bass.AP(
                                    tensor=Rsl.tensor, offset=Rsl.offset,
                                    ap=[Rsl.ap[0], [0, 8], Rsl.ap[1]])
                                nc.vector.tensor_mul(
                                    attx[:, :, sl], attx[:, :, sl], R_bsl)
                                nc.vector.tensor_add(
                                    am_a[:, :, sl], am_a[:, :, sl],
                                    attx[:, :, sl])
                            continue
                        nc.vector.tensor_mul(attx, attx, R_b)
                        if ct == 0 and g == 0:
                            nc.vector.tensor_copy(am_a, attx)
                        else:
                            nc.vector.tensor_add(
                                am_a[:, :, 0:640], am_a[:, :, 0:640],
                                attx[:, :, 0:640])
                            nc.gpsimd.tensor_add(
                                am_a[:, :, 640:T], am_a[:, :, 640:T],
                                attx[:, :, 640:T])

                # software pipeline: the k/v-proj blocks for the NEXT pair
                # sit inside and at the end of the current pair's qk/av loop —
                # they have no exp dependency, so they absorb the ACT exp lag
                # (mid-loop and across the loop seam) and the PE never stalls
                # on psum slots waiting for exps
                kc = k_proj(0)
                v_sub(0, 0)
                qk_alloc(0, kc)
                pend = {}
                for te in range(2):
                    qk_te(0, te)
                v_sub(0, 1)
                for te in range(2, 4):
                    qk_te(0, te)
                pend[1] = k_proj(1)
                for te in range(4, 8):
                    qk_te(0, te)
                v_sub(0, 2)
                v_sub(0, 3)
                for ct in range(1, 8):
                    qk_alloc(ct, pend.pop(ct))
                    for te in range(4):
                        qk_te(ct, te)
                        av_te(ct - 1, te)
                    if ct < 7:
                        pend[ct + 1] = k_proj(ct + 1)
                    if ct == 2:
                        nc.sync.dma_start(out=wp_sb, in_=wp_d[:, :, :])
                    for te in range(4, 8):
                        qk_te(ct, te)
                        av_te(ct - 1, te)
                    if 1 <= ct <= 4:
                        v_sub(1, ct - 1)
                    tail(ct - 1)
                for te in range(8):
                    av_te(7, te)
                tail(7)

                # -------- P3: output proj, then att_mean transpose ------
                # Uses the psA "pp" psum slots (freed early by the kv evacs)
                # so the partial accumulations start right after av[7] with
                # no psum bank dependency on the late y-norms. The ci=0..6
                # partials depend only on head pairs 0..6.
                def yproj_partial(pso, tt):
                    for c2 in range(2):
                        for ci in range(7):
                            nc.tensor.matmul(
                                pso[:, c2 * 512:(c2 + 1) * 512],
                                yT[:, ci, tt * 128:(tt + 1) * 128],
                                wp_sb[:, ci, c2 * 512:(c2 + 1) * 512],
                                start=(ci == 0), stop=False,
                            )

                def yproj_finish(pso, tt):
                    for c2 in range(2):
                        nc.tensor.matmul(
                            pso[:, c2 * 512:(c2 + 1) * 512],
                            yT[:, 7, tt * 128:(tt + 1) * 128],
                            wp_sb[:, 7, c2 * 512:(c2 + 1) * 512],
                            start=False, stop=(not vp_bias),
                        )
                        if vp_bias:
                            nc.tensor.matmul(
                                pso[:, c2 * 512:(c2 + 1) * 512],
                                ones_t[:, tt * 128:(tt + 1) * 128],
                                bp_row[:, c2 * 512:(c2 + 1) * 512],
                                start=False, stop=True,
                            )
                    yo = p2.tile([128, 1024], F16, tag="yo", bufs=3,
                                 name=f"yo{tt}")
                    nc.scalar.copy(yo, pso)
                    nc.sync.dma_start(out=y_d[tt * 128:(tt + 1) * 128, :],
                                      in_=yo)

                psos = {}
                for tt in range(2):
                    psos[tt] = psA.tile([128, 1024], F32, tag="pp", bufs=2,
                                        name=f"pso{tt}")
                    yproj_partial(psos[tt], tt)
                for tt in range(8):
                    if tt not in psos:
                        psos[tt] = psA.tile([128, 1024], F32, tag="pp",
                                            bufs=2, name=f"pso{tt}")
                        yproj_partial(psos[tt], tt)
                    yproj_finish(psos.pop(tt), tt)
                for tt in range(8):
                    psm = psA.tile([128, 1024], F16, tag="pp", bufs=2,
                                   name=f"psm{tt}")
                    for te in range(8):
                        nc.tensor.transpose(
                            psm[:, te * 128:(te + 1) * 128],
                            am_a[:, te, tt * 128:(tt + 1) * 128],
                            ident16,
                        )
                    mo = p2.tile([128, 1024], F16, tag="yo", bufs=3,
                                 name=f"mo{tt}")
                    # am_a already holds att/16 (exp bias fold) — plain copy
                    if tt % 2 == 0:
                        nc.scalar.copy(mo, psm)
                    else:
                        nc.vector.tensor_copy(mo, psm)
                    nc.sync.dma_start(out=am_d[tt * 128:(tt + 1) * 128, :],
                                      in_=mo)

    nc.finalize()
    return nc


def _get_nc(vp_bias=False):
    if vp_bias not in _NC:
        _NC[vp_bias] = _build(vp_bias=vp_bias)
    return _NC[vp_bias]


def _numpy_fallback(x, enc, mask, wq, bq, wk, bk, wv, bv, wp, bp):
    B, Tt, Cc = x.shape
    q = (x @ wq + bq).reshape(B, Tt, H, DH)
    k = (enc @ wk + bk).reshape(B, enc.shape[1], H, DH)
    v = (enc @ wv + bv).reshape(B, enc.shape[1], H, DH)
    att = np.einsum("bqhd,bkhd->bhqk", q, k).astype(np.float32) * SCALE
    att = np.where(mask[:, None, :, :], -np.inf, att)
    att = att - att.max(axis=-1, keepdims=True)
    att = np.exp(att)
    att = att / att.sum(axis=-1, keepdims=True)
    y = np.einsum("bhqk,bkhd->bqhd", att, v).reshape(B, Tt, Cc)
    am = att.mean(axis=1)
    y = y @ wp + bp
    return y.astype(np.float32), am.astype(np.float32)


def _sbuf_major(a):
    """[R, cols] -> [128, R//128, cols] with row index = a*128+p."""
    r, cols = a.shape
    return np.ascontiguousarray(a.reshape(r // 128, 128, cols).transpose(1, 0, 2))


def _run(inputs, trace=False):
    x = np.asarray(inputs["x"], dtype=np.float32)
    enc = np.asarray(inputs["encoder_output"], dtype=np.float32)
    mask = np.asarray(inputs["mask"])
    wq = np.asarray(inputs["Wq"], dtype=np.float32)
    wk = np.asarray(inputs["Wk"], dtype=np.float32)
    wv = np.asarray(inputs["Wv"], dtype=np.float32)
    wp = np.asarray(inputs["Wp"], dtype=np.float32)
    bq = np.asarray(inputs["bq"], dtype=np.float32).reshape(1, C)
    bk = np.asarray(inputs["bk"], dtype=np.float32).reshape(1, C)
    bv = np.asarray(inputs["bv"], dtype=np.float32).reshape(1, C)
    bp = np.asarray(inputs["bp"], dtype=np.float32).reshape(1, C)

    if mask.any():
        return _numpy_fallback(x, enc, mask,
                               wq, bq[0], wk, bk[0], wv, bv[0],
                               wp, bp[0]), None

    # Wp is pre-scaled x16: the device computes y/16 (exp bias -ln16 folds
    # the head-mean /16 into attx), so the proj weight absorbs the x16.
    w16 = {n: _sbuf_major(w.astype(np.float16))
           for n, w in (("wv", wv), ("wp", wp * np.float32(16.0)))}
    # wq/wk chunked per co-chunk / head-pair: [8, 128, 8, 128]
    wq16 = np.ascontiguousarray(
        _sbuf_major(wq.astype(np.float16))
        .reshape(128, 8, 8, 128).transpose(2, 0, 1, 3))
    wk16 = np.ascontiguousarray(
        _sbuf_major(wk.astype(np.float16))
        .reshape(128, 8, 8, 128).transpose(2, 0, 1, 3))
    bq_sb = np.ascontiguousarray(bq.reshape(8, 128).T.astype(np.float32))
    bk_sb = np.ascontiguousarray(bk.reshape(8, 128).T.astype(np.float32))
    x16 = x.astype(np.float16)
    enc16 = enc.astype(np.float16)

    nc = _get_nc(vp_bias=bool(bv.any() or bp.any()))
    in_maps = []
    encT_cache = {}
    for c in range(8):
        b, th = divmod(c, 2)
        if b not in encT_cache:
            encT_cache[b] = _sbuf_major(np.ascontiguousarray(enc16[b].T))
        in_maps.append({
            "xh": _sbuf_major(np.ascontiguousarray(
                x16[b, th * T:(th + 1) * T].T)),
            "enc": encT_cache[b],
            "wq": wq16, "wk": wk16,
            "wv": w16["wv"], "wp": w16["wp"],
            "bq": bq_sb, "bk": bk_sb,
            "bv": bv.astype(np.float16), "bp": bp.astype(np.float16),
        })
    res = run_bass_kernel_spmd(nc, in_maps, core_ids=list(range(8)),
                               trace=trace)
    B = x.shape[0]
    y = np.empty((B, 2 * T, C), np.float32)
    am = np.empty((B, 2 * T, TE), np.float32)
    for c in range(8):
        b, th = divmod(c, 2)
        y[b, th * T:(th + 1) * T] = res.results[c]["y"].astype(np.float32)
        am[b, th * T:(th + 1) * T] = res.results[c]["am"].astype(np.float32)
    return (y, am), res


def kernel(**inputs):
    out, _ = _run(inputs, trace=False)
    return out



# revision 38
# speedup vs baseline: 1.0390x; 1.0390x over previous
"""Cross-attention kernel for Trainium2, 8 NeuronCores.

Sharding: core c handles batch b=c//2 and query-half th=c%2 (1024 of 2048
query rows), all 16 heads, full T_E=1024 keys. No cross-core reduction is
needed: each core produces complete [1024, 1024] slices of both outputs.

Host-side prep (free w.r.t. the device timeline): x-half and enc are
transposed to feature-major and laid out so every DMA is contiguous
per-partition; weights are pre-arranged to SBUF order; outputs are fp16 on
device and upcast on host.

Per-core pipeline:
  P1: qT = Wq.T @ xT (fp16 [co, t]); evac via ACT w/ bias.
  P2 (per head-pair ct, software-pipelined): k-proj for the pair's 128
      columns; v-proj in N=512 matmuls (two 8-head halves x four te-pair
      subtiles, interleaved into the qk stream; 128 matmuls vs 512);
      qk per te with the two K=64 head groups packed via tile_position;
      exp on ACT with bias -ln16 -> attx = exp(s)/16 fp16 (folds the
      head-mean /16; ones col of v_aug is 16 so the av sum row is the
      true rowsum, and Wp is pre-scaled x16 host-side); av matmul (M=65)
      interleaved per-te with the NEXT pair's qk so the PE never starves;
      reciprocal (fp16, DVE) -> gpsimd partition-broadcast R; y-norm
      (DVE); att_mean acc += attx * R (DVE 0:640 + gpsimd 640:1024; the
      final head is chunked by t-slice so the P3 transposes start early).
  P3: y = yT.T @ Wp (+bp via ones row) with fp16 out; then att_mean
      transposed back on PE (plain fp16 copies; mean already folded).

mask is all-False per the input spec (fill=zeros); if a nonzero mask is ever
passed, a numpy fallback computes the exact reference instead.
"""

import sys

sys.path.insert(0, "/opt/trn_rl_repo")

import numpy as np

import concourse.bass as bass
import concourse.tile as tile
from concourse import bacc, mybir
from concourse.bass_utils import run_bass_kernel_spmd
from concourse.masks import make_identity

F32 = mybir.dt.float32
F16 = mybir.dt.float16

T = 1024   # local query rows per core
TE = 1024  # encoder tokens
C = 1024   # embed dim
H = 16     # heads
DH = 64    # head dim
SCALE = 0.125  # 1/sqrt(64)

_NC = {}


def _build(vp_bias=True):
    nc = bacc.Bacc("TRN2", target_bir_lowering=False, debug=False, num_devices=8)
    xh = nc.declare_dram_parameter("xh", [128, 8, T], F16, isOutput=False)
    enc = nc.declare_dram_parameter("enc", [128, 8, TE], F16, isOutput=False)
    wq_d = nc.declare_dram_parameter("wq", [8, 128, 8, 128], F16, isOutput=False)
    wk_d = nc.declare_dram_parameter("wk", [8, 128, 8, 128], F16, isOutput=False)
    wv_d = nc.declare_dram_parameter("wv", [128, 8, C], F16, isOutput=False)
    wp_d = nc.declare_dram_parameter("wp", [128, 8, C], F16, isOutput=False)
    bq_d = nc.declare_dram_parameter("bq", [128, 8], F32, isOutput=False)
    bk_d = nc.declare_dram_parameter("bk", [128, 8], F32, isOutput=False)
    bv_d = nc.declare_dram_parameter("bv", [1, C], F16, isOutput=False)
    bp_d = nc.declare_dram_parameter("bp", [1, C], F16, isOutput=False)
    y_d = nc.declare_dram_parameter("y", [T, C], F16, isOutput=True)
    am_d = nc.declare_dram_parameter("am", [T, TE], F16, isOutput=True)

    Exp = mybir.ActivationFunctionType.Exp
    Ident = mybir.ActivationFunctionType.Identity

    with tile.TileContext(nc) as tc:
      with tc.tile_pool(name="persist", bufs=1) as persist:
        ident16 = persist.tile([128, 128], F16)
        make_identity(nc, ident16)

        qT = persist.tile([128, 8, T], F16)       # [co%128, co//128, t]
        v_aug = persist.tile([128, 8, H, 66], F16)  # [te%128, te//128, h, .]
        yT = persist.tile([128, 8, T], F16)       # [c%128, c//8, t]
        am_a = persist.tile([128, 8, T], F16)     # [te%128, te//128, t]
        bq_sb = persist.tile([128, 8], F32)
        bk_sb = persist.tile([128, 8], F32)
        if vp_bias:
            ones_t = persist.tile([1, T], F16)    # proj bias row (lhsT)
            bv_row = persist.tile([1, C], F16)    # rhs bias row for v
            bp_row = persist.tile([1, C], F16)    # rhs bias row for proj
            ones128 = persist.tile([1, 128], F16)  # lhsT ones row for v bias

        # head-mean fold: attx is stored as exp(s)/16 (exp bias -ln16), the
        # ones column is 16 so the av sum row is the true rowsum, and Wp is
        # pre-scaled x16 host-side to undo the /16 on y.
        nln16 = persist.tile([128, 1], F32)
        nc.vector.memset(nln16, -2.772588722239781)
        nc.vector.memset(v_aug[:, :, :, 64:66], 0.0)
        nc.vector.memset(v_aug[:, :, :, 64:65], 16.0)
        if vp_bias:
            nc.vector.memset(ones_t, 1.0)
            nc.vector.memset(ones128, 1.0)

        with tc.tile_pool(name="mid", bufs=1) as mid, \
             tc.tile_pool(name="psA", bufs=2, space="PSUM") as psA:
            # ---- long-lived loads (enc/wv/wp survive into P2/P3) ----
            encT = mid.tile([128, 8, TE], F16)
            wv_sb = mid.tile([128, 8, C], F16)
            wp_sb = mid.tile([128, 8, C], F16)

            with tc.tile_pool(name="early", bufs=1) as early:
                xT = early.tile([128, 8, T], F16)
                wq_sb = early.tile([128, 8, C], F16)
                # priority order: q-proj inputs first, split for earliest
                # PE start
                wq_cs = [wq_sb[:, :, co * 128:(co + 1) * 128]
                         for co in range(8)]
                nc.sync.dma_start(out=xT[:, :, 0:512], in_=xh[:, :, 0:512])
                nc.sync.dma_start(out=wq_cs[0], in_=wq_d[0])
                nc.sync.dma_start(out=xT[:, :, 512:T], in_=xh[:, :, 512:T])
                for co in range(1, 8):
                    nc.sync.dma_start(out=wq_cs[co], in_=wq_d[co])
                nc.sync.dma_start(out=bq_sb, in_=bq_d[:, :])
                nc.sync.dma_start(out=bk_sb, in_=bk_d[:, :])
                nc.sync.dma_start(out=encT, in_=enc[:, :, :])
                nc.sync.dma_start(out=wv_sb, in_=wv_d[:, :, :])
                if vp_bias:
                    nc.sync.dma_start(out=bv_row, in_=bv_d[:, :])
                    nc.sync.dma_start(out=bp_row, in_=bp_d[:, :])
                # wp is loaded later (inside the ct loop) so the early wk
                # chunk DMAs are not queued behind this 2MB transfer

                # ---------------- P1: q projection ----------------
                for co in range(8):
                    psq = psA.tile([128, 1024], F32, tag="pp", bufs=2)
                    for t2 in range(2):
                        for ci in range(8):
                            nc.tensor.matmul(
                                psq[:, t2 * 512:(t2 + 1) * 512],
                                wq_sb[:, ci, co * 128:(co + 1) * 128],
                                xT[:, ci, t2 * 512:(t2 + 1) * 512],
                                start=(ci == 0),
                                stop=(ci == 7),
                            )
                    nc.scalar.activation(
                        out=qT[:, co, :], in_=psq, func=Ident,
                        bias=bq_sb[:, co:co + 1], scale=1.0,
                    )

            # ---------------- P2: k/v-proj + attention, pipelined --------
            with tc.tile_pool(name="p2", bufs=1) as p2:
                state = {}  # per-ct live tiles: attxs, psys, kchunk

                def yproj_partial(pso, tt):
                    # ci 0..5 only: runnable after tail(5), so the tt=0,1
                    # partials fill the PE while av(7) waits on tail(6)
                    for c2 in range(2):
                        for ci in range(6):
                            nc.tensor.matmul(
                                pso[:, c2 * 512:(c2 + 1) * 512],
                                yT[:, ci, tt * 128:(tt + 1) * 128],
                                wp_sb[:, ci, c2 * 512:(c2 + 1) * 512],
                                start=(ci == 0), stop=False,
                            )

                def k_proj_start(ct):
                    # k projection for pair ct's 128 co-columns; the matmuls
                    # are emitted later in 4 chunks interleaved into the qk
                    # stream so they absorb exp lag exactly where it occurs
                    wk_ct = p2.tile([128, 8, 128], F16, tag="wk", bufs=2,
                                    name=f"wk{ct}")
                    nc.sync.dma_start(out=wk_ct, in_=wk_d[ct, :, :, :])
                    kc = p2.tile([128, TE], F16, tag="kc", bufs=2,
                                 name=f"kc{ct}")
                    psk = psA.tile([128, 1024], F32, tag="pp", bufs=2,
                                   name=f"psk{ct}")
                    return (ct, wk_ct, kc, psk)

                def k_proj_chunk(kp, i):
                    ct, wk_ct, kc, psk = kp
                    t2, half = divmod(i, 2)
                    for ci in range(half * 4, half * 4 + 4):
                        nc.tensor.matmul(
                            psk[:, t2 * 512:(t2 + 1) * 512],
                            wk_ct[:, ci, :],
                            encT[:, ci, t2 * 512:(t2 + 1) * 512],
                            start=(ci == 0),
                            stop=(ci == 7),
                        )
                    if i == 3:
                        nc.scalar.activation(
                            out=kc, in_=psk, func=Ident,
                            bias=bk_sb[:, ct:ct + 1], scale=1.0,
                        )

                def k_proj(ct):
                    kp = k_proj_start(ct)
                    for i in range(4):
                        k_proj_chunk(kp, i)
                    return kp[2]

                def v_sub_start(half, tp):
                    # v projection for co-half `half` (8 heads), te chunks
                    # 2*tp and 2*tp+1, with full-width N=512 matmuls (one
                    # accumulation group per 512-col psum bank); emitted in
                    # 2 chunks
                    psv = psA.tile([128, 2, 8, 64], F32, tag="pp", bufs=2,
                                   name=f"psv{half}_{tp}")
                    return (half, tp, psv)

                def v_sub_chunk(vs, j):
                    half, tp, psv = vs
                    te = 2 * tp + j
                    for ci in range(8):
                        nc.tensor.matmul(
                            psv[:, j, :, :],
                            encT[:, ci, te * 128:(te + 1) * 128],
                            wv_sb[:, ci, half * 512:(half + 1) * 512],
                            start=(ci == 0),
                            stop=(ci == 7 and not vp_bias),
                        )
                    if vp_bias:
                        nc.tensor.matmul(
                            psv[:, j, :, :],
                            ones128,
                            bv_row[:, half * 512:(half + 1) * 512],
                            start=False, stop=True,
                        )
                    if j == 1:
                        nc.scalar.copy(
                            v_aug[:, 2 * tp:2 * tp + 2,
                                  8 * half:8 * half + 8, 0:64], psv)

                def v_sub(half, tp):
                    vs = v_sub_start(half, tp)
                    v_sub_chunk(vs, 0)
                    v_sub_chunk(vs, 1)

                def qk_alloc(ct, kc):
                    attxs = [p2.tile([128, 8, T], F16, tag="attx", bufs=4,
                                     name=f"attx{ct}g{g}") for g in range(2)]
                    psys = [psA.tile([65, 1024], F32, tag="py", bufs=2,
                                     name=f"psy{ct}g{g}") for g in range(2)]
                    state[ct] = (attxs, psys, kc)

                def qk_te(ct, te):
                    attxs, _, kc = state[ct]
                    psas = [psA.tile([128, 1024], F32, tag="pp", bufs=2,
                                     name=f"psa{ct}t{te}g{g}") for g in range(2)]
                    for t2 in range(2):
                        for g, hp in enumerate((0, 64)):
                            nc.tensor.matmul(
                                psas[g][:, t2 * 512:(t2 + 1) * 512],
                                kc[hp:hp + 64, te * 128:(te + 1) * 128],
                                qT[hp:hp + 64, ct, t2 * 512:(t2 + 1) * 512],
                                start=True, stop=True,
                                tile_position=(hp, 0),
                            )
                    for g in range(2):
                        nc.scalar.activation(
                            out=attxs[g][:, te, :], in_=psas[g],
                            func=Exp, scale=SCALE, bias=nln16[:, 0:1],
                        )

                def av_te(ct, te):
                    attxs, psys, _ = state[ct]
                    for g, hp in enumerate((0, 64)):
                        h = 2 * ct + g
                        for t2 in range(2):
                            nc.tensor.matmul(
                                psys[g][:, t2 * 512:(t2 + 1) * 512],
                                v_aug[:, te, h, 0:65],
                                attxs[g][:, te, t2 * 512:(t2 + 1) * 512],
                                start=(te == 0), stop=(te == 7),
                            )

                def tail(ct):
                    attxs, psys, _ = state.pop(ct)
                    Rs = []
                    # recips + broadcasts + ynorms first: frees the psy psum
                    # slots early so the next pair's av is not gated on the
                    # att_mean accumulation below
                    for g in range(2):
                        r16 = p2.tile([1, T], F16, tag="r16", bufs=2,
                                      name=f"r{ct}g{g}")
                        with nc.allow_low_precision("softmax recip fp16"):
                            nc.vector.reciprocal(r16, psys[g][64:65, :])
                        R = p2.tile([128, T], F16, tag="R", bufs=2,
                                    name=f"R{ct}g{g}")
                        nc.gpsimd.partition_broadcast(R, r16)
                        Rs.append(R)
                    for g, hp in enumerate((0, 64)):
                        nc.vector.tensor_mul(
                            yT[hp:hp + 64, ct, :], psys[g][0:64, :],
                            Rs[g][0:64, :]
                        )
                    # att_mean accumulator += attx * R (normalize attx
                    # in place, then accumulate; half of the g1 add runs on
                    # the mostly-idle Pool engine to balance DVE).  The final
                    # head (ct=7 g=1) is chunked by t so the P3 am transposes
                    # can start per-slice instead of waiting for one big add.
                    for g in range(2):
                        attx, R = attxs[g], Rs[g]
                        R_b = bass.AP(
                            tensor=R.tensor, offset=R.offset,
                            ap=[R.ap[0], [0, 8], R.ap[1]],
                        )
                        if ct == 7 and g == 1:
                            for tt in range(8):
                                sl = slice(tt * 128, (tt + 1) * 128)
                                Rsl = R[:, sl]
                                R_bsl = bass.AP(
                                    tensor=Rsl.tensor, offset=Rsl.offset,
                                    ap=[Rsl.ap[0], [0, 8], Rsl.ap[1]])
                                nc.vector.tensor_mul(
                                    attx[:, :, sl], attx[:, :, sl], R_bsl)
                                nc.vector.tensor_add(
                                    am_a[:, :, sl], am_a[:, :, sl],
                                    attx[:, :, sl])
                            continue
                        nc.vector.tensor_mul(attx, attx, R_b)
                        if ct == 0 and g == 0:
                            nc.vector.tensor_copy(am_a, attx)
                        else:
                            nc.vector.tensor_add(
                                am_a[:, :, 0:640], am_a[:, :, 0:640],
                                attx[:, :, 0:640])
                            nc.gpsimd.tensor_add(
                                am_a[:, :, 640:T], am_a[:, :, 640:T],
                                attx[:, :, 640:T])

                # software pipeline: the k/v-proj matmuls for the NEXT pair
                # are emitted in small chunks after each qk step — they have
                # no exp dependency, so they absorb the ACT exp lag at every
                # te instead of in lumps, keeping the PE off the psum-slot
                # wait for exps
                kc = k_proj(0)
                v_sub(0, 0)
                qk_alloc(0, kc)
                pend = {}
                # ct=0 fillers: v(0,1), v(0,2), v(0,3) and kproj(1) chunks
                kp1 = None
                for te in range(8):
                    qk_te(0, te)
                    if te == 0:
                        vs = v_sub_start(0, 1)
                        v_sub_chunk(vs, 0)
                    elif te == 1:
                        v_sub_chunk(vs, 1)
                    elif te == 2:
                        kp1 = k_proj_start(1)
                        k_proj_chunk(kp1, 0)
                        k_proj_chunk(kp1, 1)
                    elif te == 3:
                        k_proj_chunk(kp1, 2)
                        k_proj_chunk(kp1, 3)
                    elif te in (4, 6):
                        vs = v_sub_start(0, te // 2)
                        v_sub_chunk(vs, 0)
                    else:
                        v_sub_chunk(vs, 1)
                pend[1] = kp1[2]
                for ct in range(1, 8):
                    qk_alloc(ct, pend.pop(ct))
                    kp = k_proj_start(ct + 1) if ct < 7 else None
                    # pairs with v_sub filler put kproj in the first half;
                    # pairs 5-6 spread kproj across both halves so the
                    # second half is not filler-starved
                    kslots = (0, 1, 2, 3) if ct <= 4 else (1, 3, 5, 7)
                    for te in range(4):
                        qk_te(ct, te)
                        av_te(ct - 1, te)
                        if kp is not None and te in kslots:
                            k_proj_chunk(kp, kslots.index(te))
                    if ct == 2:
                        nc.sync.dma_start(out=wp_sb, in_=wp_d[:, :, :])
                    vs = v_sub_start(1, ct - 1) if 1 <= ct <= 4 else None
                    for te in range(4, 8):
                        qk_te(ct, te)
                        av_te(ct - 1, te)
                        if vs is not None and te in (5, 7):
                            v_sub_chunk(vs, (te - 5) // 2)
                        if kp is not None and te in kslots:
                            k_proj_chunk(kp, kslots.index(te))
                    if kp is not None:
                        pend[ct + 1] = kp[2]
                    tail(ct - 1)
                # tt=0,1 yproj partials (ci 0..5) fill the PE queue while
                # av(7) waits for tail(6)'s psum release
                _early_psos = {}
                for tt in range(2):
                    _early_psos[tt] = psA.tile([128, 1024], F32, tag="pp",
                                               bufs=2, name=f"pso{tt}")
                    yproj_partial(_early_psos[tt], tt)
                for te in range(8):
                    av_te(7, te)
                tail(7)

                # -------- P3: output proj, then att_mean transpose ------
                # Uses the psA "pp" psum slots (freed early by the kv evacs)
                # so the partial accumulations start right after av[7] with
                # no psum bank dependency on the late y-norms. The ci=0..6
                # partials depend only on head pairs 0..6.
                def yproj_finish(pso, tt):
                    for c2 in range(2):
                        for ci in (6, 7):
                            nc.tensor.matmul(
                                pso[:, c2 * 512:(c2 + 1) * 512],
                                yT[:, ci, tt * 128:(tt + 1) * 128],
                                wp_sb[:, ci, c2 * 512:(c2 + 1) * 512],
                                start=False,
                                stop=(ci == 7 and not vp_bias),
                            )
                        if vp_bias:
                            nc.tensor.matmul(
                                pso[:, c2 * 512:(c2 + 1) * 512],
                                ones_t[:, tt * 128:(tt + 1) * 128],
                                bp_row[:, c2 * 512:(c2 + 1) * 512],
                                start=False, stop=True,
                            )
                    yo = p2.tile([128, 1024], F16, tag="yo", bufs=3,
                                 name=f"yo{tt}")
                    nc.scalar.copy(yo, pso)
                    nc.sync.dma_start(out=y_d[tt * 128:(tt + 1) * 128, :],
                                      in_=yo)

                psos = {}
                for tt in range(2):
                    psos[tt] = psA.tile([128, 1024], F32, tag="pp", bufs=2,
                                        name=f"pso{tt}")
                    yproj_partial(psos[tt], tt)
                for tt in range(8):
                    if tt not in psos:
                        psos[tt] = psA.tile([128, 1024], F32, tag="pp",
                                            bufs=2, name=f"pso{tt}")
                        yproj_partial(psos[tt], tt)
                    yproj_finish(psos.pop(tt), tt)
                for tt in range(8):
                    psm = psA.tile([128, 1024], F16, tag="pp", bufs=2,
                                   name=f"psm{tt}")
                    for te in range(8):
                        nc.tensor.transpose(
                            psm[:, te * 128:(te + 1) * 128],
                            am_a[:, te, tt * 128:(tt + 1) * 128],
                            ident16,
                        )
                    mo = p2.tile([128, 1024], F16, tag="yo", bufs=3,
                                 name=f"mo{tt}")
                    # am_a already holds att/16 (exp bias fold) — plain copy
                    if tt % 2 == 0:
                        nc.scalar.copy(mo, psm)
                    else:
                        nc.vector.tensor_copy(mo, psm)
                    nc.sync.dma_start(out=am_d[tt * 128:(tt + 1) * 128, :],
                                      in_=mo)

    nc.finalize()
    return nc


def _get_nc(vp_bias=False):
    if vp_bias not in _NC:
        _NC[vp_bias] = _build(vp_bias=vp_bias)
    return _NC[vp_bias]


def _numpy_fallback(x, enc, mask, wq, bq, wk, bk, wv, bv, wp, bp):
    B, Tt, Cc = x.shape
    q = (x @ wq + bq).reshape(B, Tt, H, DH)
    k = (enc @ wk + bk).reshape(B, enc.shape[1], H, DH)
    v = (enc @ wv + bv).reshape(B, enc.shape[1], H, DH)
    att = np.einsum("bqhd,bkhd->bhqk", q, k).astype(np.float32) * SCALE
    att = np.where(mask[:, None, :, :], -np.inf, att)
    att = att - att.max(axis=-1, keepdims=True)
    att = np.exp(att)
    att = att / att.sum(axis=-1, keepdims=True)
    y = np.einsum("bhqk,bkhd->bqhd", att, v).reshape(B, Tt, Cc)
    am = att.mean(axis=1)
    y = y @ wp + bp
    return y.astype(np.float32), am.astype(np.float32)


def _sbuf_major(a):
    """[R, cols] -> [128, R//128, cols] with row index = a*128+p."""
    r, cols = a.shape
    return np.ascontiguousarray(a.reshape(r // 128, 128, cols).transpose(1, 0, 2))


def _run(inputs, trace=False):
    x = np.asarray(inputs["x"], dtype=np.float32)
    enc = np.asarray(inputs["encoder_output"], dtype=np.float32)
    mask = np.asarray(inputs["mask"])
    wq = np.asarray(inputs["Wq"], dtype=np.float32)
    wk = np.asarray(inputs["Wk"], dtype=np.float32)
    wv = np.asarray(inputs["Wv"], dtype=np.float32)
    wp = np.asarray(inputs["Wp"], dtype=np.float32)
    bq = np.asarray(inputs["bq"], dtype=np.float32).reshape(1, C)
    bk = np.asarray(inputs["bk"], dtype=np.float32).reshape(1, C)
    bv = np.asarray(inputs["bv"], dtype=np.float32).reshape(1, C)
    bp = np.asarray(inputs["bp"], dtype=np.float32).reshape(1, C)

    if mask.any():
        return _numpy_fallback(x, enc, mask,
                               wq, bq[0], wk, bk[0], wv, bv[0],
                               wp, bp[0]), None

    # Wp is pre-scaled x16: the device computes y/16 (exp bias -ln16 folds
    # the head-mean /16 into attx), so the proj weight absorbs the x16.
    w16 = {n: _sbuf_major(w.astype(np.float16))
           for n, w in (("wv", wv), ("wp", wp * np.float32(16.0)))}
    # wq/wk chunked per co-chunk / head-pair: [8, 128, 8, 128]
    wq16 = np.ascontiguousarray(
        _sbuf_major(wq.astype(np.float16))
        .reshape(128, 8, 8, 128).transpose(2, 0, 1, 3))
    wk16 = np.ascontiguousarray(
        _sbuf_major(wk.astype(np.float16))
        .reshape(128, 8, 8, 128).transpose(2, 0, 1, 3))
    bq_sb = np.ascontiguousarray(bq.reshape(8, 128).T.astype(np.float32))
    bk_sb = np.ascontiguousarray(bk.reshape(8, 128).T.astype(np.float32))
    x16 = x.astype(np.float16)
    enc16 = enc.astype(np.float16)

    nc = _get_nc(vp_bias=bool(bv.any() or bp.any()))
    in_maps = []
    encT_cache = {}
    for c in range(8):
        b, th = divmod(c, 2)
        if b not in encT_cache:
            encT_cache[b] = _sbuf_major(np.ascontiguousarray(enc16[b].T))
        in_maps.append({
            "xh": _sbuf_major(np.ascontiguousarray(
                x16[b, th * T:(th + 1) * T].T)),
            "enc": encT_cache[b],
            "wq": wq16, "wk": wk16,
            "wv": w16["wv"], "wp": w16["wp"],
            "bq": bq_sb, "bk": bk_sb,
            "bv": bv.astype(np.float16), "bp": bp.astype(np.float16),
        })
    res = run_bass_kernel_spmd(nc, in_maps, core_ids=list(range(8)),
                               trace=trace)
    B = x.shape[0]
    y = np.empty((B, 2 * T, C), np.float32)
    am = np.empty((B, 2 * T, TE), np.float32)
    for c in range(8):
        b, th = divmod(c, 2)
        y[b, th * T:(th + 1) * T] = res.results[c]["y"].astype(np.float32)
        am[b, th * T:(th + 1) * T] = res.results[c]["am"].astype(np.float32)
    return (y, am), res


def kernel(**inputs):
    out, _ = _run(inputs, trace=False)
    return out



# revision 54
# speedup vs baseline: 1.0619x; 1.0221x over previous
"""Cross-attention kernel for Trainium2, 8 NeuronCores.

Sharding: core c handles batch b=c//2 and query-half th=c%2 (1024 of 2048
query rows), all 16 heads, full T_E=1024 keys. No cross-core reduction is
needed: each core produces complete [1024, 1024] slices of both outputs.

Host-side prep (free w.r.t. the device timeline): x-half and enc are
transposed to feature-major and laid out so every DMA is contiguous
per-partition; weights are pre-arranged to SBUF order; outputs are fp16 on
device and upcast on host.

Per-core pipeline:
  P1: qT = Wq.T @ xT (fp16 [co, t]); evac via ACT w/ bias.
  P2 (per head-pair ct, software-pipelined): k-proj for the pair's 128
      columns and v-proj (N=512 matmuls, 8-head halves x te-pair subtiles)
      are emitted in small chunks after each qk step, absorbing the ACT
      exp lag at every te instead of in lumps;
      qk per te with the two K=64 head groups packed via tile_position;
      exp on ACT with bias -ln16 -> attx = exp(s)/16 fp16 (folds the
      head-mean /16; ones col of v_aug is 16 so the av sum row is the
      true rowsum, and Wp is pre-scaled x16 host-side); av matmul (M=65)
      interleaved per-te with the NEXT pair's qk so the PE never starves;
      reciprocal (fp16, DVE) -> gpsimd partition-broadcast R; y-norm
      (DVE); att_mean acc += attx * R (DVE 0:640 + gpsimd 640:1024; the
      final head is chunked by t-slice so the P3 transposes start early).
  P3: y = yT.T @ Wp (+bp via ones row) with fp16 out; the ci 0-5 yproj
      partials for the first two t-chunks are emitted before av(7) to fill
      the PE while tail(6) releases its psum; then att_mean is transposed
      back on PE (plain fp16 copies; mean already folded).

mask is all-False per the input spec (fill=zeros); if a nonzero mask is ever
passed, a numpy fallback computes the exact reference instead.
"""

import sys

sys.path.insert(0, "/opt/trn_rl_repo")

import numpy as np

import concourse.bass as bass
import concourse.tile as tile
from concourse import bacc, mybir
from concourse.bass_utils import run_bass_kernel_spmd
from concourse.masks import make_identity

F32 = mybir.dt.float32
F16 = mybir.dt.float16

T = 1024   # local query rows per core
TE = 1024  # encoder tokens
C = 1024   # embed dim
H = 16     # heads
DH = 64    # head dim
SCALE = 0.125  # 1/sqrt(64)

_NC = {}


def _build(vp_bias=True):
    nc = bacc.Bacc("TRN2", target_bir_lowering=False, debug=False, num_devices=8)
    xh = nc.declare_dram_parameter("xh", [128, 8, T], F16, isOutput=False)
    enc = nc.declare_dram_parameter("enc", [128, 8, TE], F16, isOutput=False)
    wq_d = nc.declare_dram_parameter("wq", [8, 128, 8, 128], F16, isOutput=False)
    wk_d = nc.declare_dram_parameter("wk", [8, 128, 8, 128], F16, isOutput=False)
    wv_d = nc.declare_dram_parameter("wv", [128, 8, C], F16, isOutput=False)
    wp_d = nc.declare_dram_parameter("wp", [128, 8, C], F16, isOutput=False)
    bq_d = nc.declare_dram_parameter("bq", [128, 8], F32, isOutput=False)
    bk_d = nc.declare_dram_parameter("bk", [128, 8], F32, isOutput=False)
    bv_d = nc.declare_dram_parameter("bv", [1, C], F16, isOutput=False)
    bp_d = nc.declare_dram_parameter("bp", [1, C], F16, isOutput=False)
    y_d = nc.declare_dram_parameter("y", [T, C], F16, isOutput=True)
    am_d = nc.declare_dram_parameter("am", [T, TE], F16, isOutput=True)

    Exp = mybir.ActivationFunctionType.Exp
    Ident = mybir.ActivationFunctionType.Identity

    with tile.TileContext(nc) as tc:
      with tc.tile_pool(name="persist", bufs=1) as persist:
        ident16 = persist.tile([128, 128], F16)
        make_identity(nc, ident16)

        qT = persist.tile([128, 8, T], F16)       # [co%128, co//128, t]
        v_aug = persist.tile([128, 8, H, 66], F16)  # [te%128, te//128, h, .]
        yT = persist.tile([128, 8, T], F16)       # [c%128, c//8, t]
        am_a = persist.tile([128, 8, T], F16)     # [te%128, te//128, t]
        bq_sb = persist.tile([128, 8], F32)
        bk_sb = persist.tile([128, 8], F32)
        if vp_bias:
            ones_t = persist.tile([1, T], F16)    # proj bias row (lhsT)
            bv_row = persist.tile([1, C], F16)    # rhs bias row for v
            bp_row = persist.tile([1, C], F16)    # rhs bias row for proj
            ones128 = persist.tile([1, 128], F16)  # lhsT ones row for v bias

        # head-mean fold: attx is stored as exp(s)/16 (exp bias -ln16), the
        # ones column is 16 so the av sum row is the true rowsum, and Wp is
        # pre-scaled x16 host-side to undo the /16 on y.
        nln16 = persist.tile([128, 1], F32)
        nc.vector.memset(nln16, -2.772588722239781)
        nc.vector.memset(v_aug[:, :, :, 64:66], 0.0)
        nc.vector.memset(v_aug[:, :, :, 64:65], 16.0)
        if vp_bias:
            nc.vector.memset(ones_t, 1.0)
            nc.vector.memset(ones128, 1.0)

        with tc.tile_pool(name="mid", bufs=1) as mid, \
             tc.tile_pool(name="psA", bufs=2, space="PSUM") as psA:
            # ---- long-lived loads (enc/wv/wp survive into P2/P3) ----
            encT = mid.tile([128, 8, TE], F16)
            wv_sb = mid.tile([128, 8, C], F16)
            wp_sb = mid.tile([128, 8, C], F16)

            with tc.tile_pool(name="early", bufs=1) as early:
                xT = early.tile([128, 8, T], F16)
                wq_sb = early.tile([128, 8, C], F16)
                # priority order: q-proj inputs first, split for earliest
                # PE start
                wq_cs = [wq_sb[:, :, co * 128:(co + 1) * 128]
                         for co in range(8)]
                nc.sync.dma_start(out=xT[:, :, 0:512], in_=xh[:, :, 0:512])
                nc.sync.dma_start(out=wq_cs[0], in_=wq_d[0])
                nc.sync.dma_start(out=xT[:, :, 512:T], in_=xh[:, :, 512:T])
                for co in range(1, 8):
                    nc.sync.dma_start(out=wq_cs[co], in_=wq_d[co])
                nc.sync.dma_start(out=bq_sb, in_=bq_d[:, :])
                nc.sync.dma_start(out=bk_sb, in_=bk_d[:, :])
                nc.sync.dma_start(out=encT, in_=enc[:, :, :])
                # wv in halves: the first 8 heads' columns arrive in time
                # for v_sub(0,2/3) without waiting the full 2MB transfer
                nc.sync.dma_start(out=wv_sb[:, :, 0:512], in_=wv_d[:, :, 0:512])
                nc.sync.dma_start(out=wv_sb[:, :, 512:C], in_=wv_d[:, :, 512:C])
                if vp_bias:
                    nc.sync.dma_start(out=bv_row, in_=bv_d[:, :])
                    nc.sync.dma_start(out=bp_row, in_=bp_d[:, :])
                # wp is loaded later (inside the ct loop) so the early wk
                # chunk DMAs are not queued behind this 2MB transfer

                # ---------------- P1: q projection ----------------
                for co in range(8):
                    psq = psA.tile([128, 1024], F32, tag="pp", bufs=2)
                    for t2 in range(2):
                        for ci in range(8):
                            nc.tensor.matmul(
                                psq[:, t2 * 512:(t2 + 1) * 512],
                                wq_sb[:, ci, co * 128:(co + 1) * 128],
                                xT[:, ci, t2 * 512:(t2 + 1) * 512],
                                start=(ci == 0),
                                stop=(ci == 7),
                            )
                    nc.scalar.activation(
                        out=qT[:, co, :], in_=psq, func=Ident,
                        bias=bq_sb[:, co:co + 1], scale=1.0,
                    )

            # ---------------- P2: k/v-proj + attention, pipelined --------
            with tc.tile_pool(name="p2", bufs=1) as p2:
                state = {}  # per-ct live tiles: attxs, psys, kchunk

                def yproj_partial(pso, tt):
                    # ci 0..5 only: runnable after tail(5), so the tt=0,1
                    # partials fill the PE while av(7) waits on tail(6)
                    for c2 in range(2):
                        for ci in range(6):
                            nc.tensor.matmul(
                                pso[:, c2 * 512:(c2 + 1) * 512],
                                yT[:, ci, tt * 128:(tt + 1) * 128],
                                wp_sb[:, ci, c2 * 512:(c2 + 1) * 512],
                                start=(ci == 0), stop=False,
                            )

                def k_proj_start(ct):
                    # k projection for pair ct's 128 co-columns; the matmuls
                    # are emitted later in 4 chunks interleaved into the qk
                    # stream so they absorb exp lag exactly where it occurs
                    wk_ct = p2.tile([128, 8, 128], F16, tag="wk", bufs=2,
                                    name=f"wk{ct}")
                    nc.sync.dma_start(out=wk_ct, in_=wk_d[ct, :, :, :])
                    kc = p2.tile([128, TE], F16, tag="kc", bufs=2,
                                 name=f"kc{ct}")
                    psk = psA.tile([128, 1024], F32, tag="pp", bufs=2,
                                   name=f"psk{ct}")
                    return (ct, wk_ct, kc, psk)

                def k_proj_chunk(kp, i):
                    ct, wk_ct, kc, psk = kp
                    t2, half = divmod(i, 2)
                    for ci in range(half * 4, half * 4 + 4):
                        nc.tensor.matmul(
                            psk[:, t2 * 512:(t2 + 1) * 512],
                            wk_ct[:, ci, :],
                            encT[:, ci, t2 * 512:(t2 + 1) * 512],
                            start=(ci == 0),
                            stop=(ci == 7),
                        )
                    if i == 3:
                        nc.scalar.activation(
                            out=kc, in_=psk, func=Ident,
                            bias=bk_sb[:, ct:ct + 1], scale=1.0,
                        )

                def k_proj(ct):
                    kp = k_proj_start(ct)
                    for i in range(4):
                        k_proj_chunk(kp, i)
                    return kp[2]

                def v_sub_start(half, tp):
                    # v projection for co-half `half` (8 heads), te chunks
                    # 2*tp and 2*tp+1, with full-width N=512 matmuls (one
                    # accumulation group per 512-col psum bank); emitted in
                    # 2 chunks
                    psv = psA.tile([128, 2, 8, 64], F32, tag="pp", bufs=2,
                                   name=f"psv{half}_{tp}")
                    return (half, tp, psv)

                def v_sub_chunk(vs, j):
                    half, tp, psv = vs
                    te = 2 * tp + j
                    for ci in range(8):
                        nc.tensor.matmul(
                            psv[:, j, :, :],
                            encT[:, ci, te * 128:(te + 1) * 128],
                            wv_sb[:, ci, half * 512:(half + 1) * 512],
                            start=(ci == 0),
                            stop=(ci == 7 and not vp_bias),
                        )
                    if vp_bias:
                        nc.tensor.matmul(
                            psv[:, j, :, :],
                            ones128,
                            bv_row[:, half * 512:(half + 1) * 512],
                            start=False, stop=True,
                        )
                    if j == 1:
                        nc.scalar.copy(
                            v_aug[:, 2 * tp:2 * tp + 2,
                                  8 * half:8 * half + 8, 0:64], psv)

                def v_sub(half, tp):
                    vs = v_sub_start(half, tp)
                    v_sub_chunk(vs, 0)
                    v_sub_chunk(vs, 1)

                def qk_alloc(ct, kc):
                    attxs = [p2.tile([128, 8, T], F16, tag="attx", bufs=4,
                                     name=f"attx{ct}g{g}") for g in range(2)]
                    psys = [psA.tile([65, 1024], F32, tag="py", bufs=2,
                                     name=f"psy{ct}g{g}") for g in range(2)]
                    state[ct] = (attxs, psys, kc)

                def qk_te(ct, te):
                    attxs, _, kc = state[ct]
                    psas = [psA.tile([128, 1024], F32, tag="pp", bufs=2,
                                     name=f"psa{ct}t{te}g{g}") for g in range(2)]
                    for t2 in range(2):
                        for g, hp in enumerate((0, 64)):
                            nc.tensor.matmul(
                                psas[g][:, t2 * 512:(t2 + 1) * 512],
                                kc[hp:hp + 64, te * 128:(te + 1) * 128],
                                qT[hp:hp + 64, ct, t2 * 512:(t2 + 1) * 512],
                                start=True, stop=True,
                                tile_position=(hp, 0),
                            )
                    for g in range(2):
                        nc.scalar.activation(
                            out=attxs[g][:, te, :], in_=psas[g],
                            func=Exp, scale=SCALE, bias=nln16[:, 0:1],
                        )

                def av_te(ct, te):
                    attxs, psys, _ = state[ct]
                    for g, hp in enumerate((0, 64)):
                        h = 2 * ct + g
                        for t2 in range(2):
                            nc.tensor.matmul(
                                psys[g][:, t2 * 512:(t2 + 1) * 512],
                                v_aug[:, te, h, 0:65],
                                attxs[g][:, te, t2 * 512:(t2 + 1) * 512],
                                start=(te == 0), stop=(te == 7),
                            )

                def tail(ct):
                    attxs, psys, _ = state.pop(ct)
                    Rs = []
                    # recips + broadcasts + ynorms first: frees the psy psum
                    # slots early so the next pair's av is not gated on the
                    # att_mean accumulation below
                    for g in range(2):
                        r16 = p2.tile([1, T], F16, tag="r16", bufs=2,
                                      name=f"r{ct}g{g}")
                        with nc.allow_low_precision("softmax recip fp16"):
                            nc.vector.reciprocal(r16, psys[g][64:65, :])
                        R = p2.tile([128, T], F16, tag="R", bufs=2,
                                    name=f"R{ct}g{g}")
                        nc.gpsimd.partition_broadcast(R, r16)
                        Rs.append(R)
                    for g, hp in enumerate((0, 64)):
                        nc.vector.tensor_mul(
                            yT[hp:hp + 64, ct, :], psys[g][0:64, :],
                            Rs[g][0:64, :]
                        )
                    # att_mean accumulator += attx * R (normalize attx
                    # in place, then accumulate; half of the g1 add runs on
                    # the mostly-idle Pool engine to balance DVE).  The final
                    # head (ct=7 g=1) is chunked by t so the P3 am transposes
                    # can start per-slice instead of waiting for one big add.
                    for g in range(2):
                        attx, R = attxs[g], Rs[g]
                        R_b = bass.AP(
                            tensor=R.tensor, offset=R.offset,
                            ap=[R.ap[0], [0, 8], R.ap[1]],
                        )
                        if ct == 7:
                            continue  # both g handled per t-slice below
                        nc.vector.tensor_mul(attx, attx, R_b)
                        if ct == 0 and g == 0:
                            nc.vector.tensor_copy(am_a, attx)
                        else:
                            nc.vector.tensor_add(
                                am_a[:, :, 0:768], am_a[:, :, 0:768],
                                attx[:, :, 0:768])
                            nc.gpsimd.tensor_add(
                                am_a[:, :, 768:T], am_a[:, :, 768:T],
                                attx[:, :, 768:T])
                    if ct == 7:
                        # final pair fully per t-slice: each P3 transpose
                        # starts as soon as its slice's accumulation is done
                        for tt in range(8):
                            sl = slice(tt * 128, (tt + 1) * 128)
                            for g in range(2):
                                Rsl = Rs[g][:, sl]
                                R_bsl = bass.AP(
                                    tensor=Rsl.tensor, offset=Rsl.offset,
                                    ap=[Rsl.ap[0], [0, 8], Rsl.ap[1]])
                                nc.vector.tensor_mul(
                                    attxs[g][:, :, sl], attxs[g][:, :, sl],
                                    R_bsl)
                            nc.vector.tensor_add(
                                am_a[:, :, sl], am_a[:, :, sl],
                                attxs[0][:, :, sl])
                            nc.vector.tensor_add(
                                am_a[:, :, sl], am_a[:, :, sl],
                                attxs[1][:, :, sl])

                # software pipeline: the k/v-proj matmuls for the NEXT pair
                # are emitted in small chunks after each qk step — they have
                # no exp dependency, so they absorb the ACT exp lag at every
                # te instead of in lumps, keeping the PE off the psum-slot
                # wait for exps
                kc = k_proj(0)
                v_sub(0, 0)
                qk_alloc(0, kc)
                pend = {}
                # ct=0 fillers: v(0,1), v(0,2), v(0,3) and kproj(1) chunks
                kp1 = None
                for te in range(8):
                    qk_te(0, te)
                    if te == 0:
                        kp1 = k_proj_start(1)
                        vs = v_sub_start(0, 1)
                        v_sub_chunk(vs, 0)
                    elif te == 1:
                        v_sub_chunk(vs, 1)
                    elif te == 2:
                        k_proj_chunk(kp1, 0)
                        k_proj_chunk(kp1, 1)
                    elif te == 3:
                        k_proj_chunk(kp1, 2)
                        k_proj_chunk(kp1, 3)
                    elif te in (4, 6):
                        vs = v_sub_start(0, te // 2)
                        v_sub_chunk(vs, 0)
                    else:
                        v_sub_chunk(vs, 1)
                pend[1] = kp1[2]
                for ct in range(1, 8):
                    qk_alloc(ct, pend.pop(ct))
                    kp = k_proj_start(ct + 1) if ct < 7 else None
                    for te in range(4):
                        qk_te(ct, te)
                        av_te(ct - 1, te)
                        if kp is not None:
                            k_proj_chunk(kp, te)
                    if kp is not None:
                        pend[ct + 1] = kp[2]
                    if ct == 2:
                        nc.sync.dma_start(out=wp_sb, in_=wp_d[:, :, :])
                    vs = v_sub_start(1, ct - 1) if 1 <= ct <= 4 else None
                    for te in range(4, 8):
                        qk_te(ct, te)
                        av_te(ct - 1, te)
                        if vs is not None and te in (5, 7):
                            v_sub_chunk(vs, (te - 5) // 2)
                    tail(ct - 1)
                # tt=0,1 yproj partials (ci 0..5) fill the PE queue while
                # av(7) waits for tail(6)'s psum release
                _early_psos = {}
                for tt in range(2):
                    _early_psos[tt] = psA.tile([128, 1024], F32, tag="pp",
                                               bufs=2, name=f"pso{tt}")
                    yproj_partial(_early_psos[tt], tt)
                for te in range(8):
                    av_te(7, te)
                tail(7)

                # -------- P3: output proj, then att_mean transpose ------
                # Uses the psA "pp" psum slots (freed early by the kv evacs)
                # so the partial accumulations start right after av[7] with
                # no psum bank dependency on the late y-norms. The ci=0..6
                # partials depend only on head pairs 0..6.
                def yproj_finish(pso, tt):
                    for c2 in range(2):
                        for ci in (6, 7):
                            nc.tensor.matmul(
                                pso[:, c2 * 512:(c2 + 1) * 512],
                                yT[:, ci, tt * 128:(tt + 1) * 128],
                                wp_sb[:, ci, c2 * 512:(c2 + 1) * 512],
                                start=False,
                                stop=(ci == 7 and not vp_bias),
                            )
                        if vp_bias:
                            nc.tensor.matmul(
                                pso[:, c2 * 512:(c2 + 1) * 512],
                                ones_t[:, tt * 128:(tt + 1) * 128],
                                bp_row[:, c2 * 512:(c2 + 1) * 512],
                                start=False, stop=True,
                            )
                    yo = p2.tile([128, 1024], F16, tag="yo", bufs=3,
                                 name=f"yo{tt}")
                    nc.scalar.copy(yo, pso)
                    nc.sync.dma_start(out=y_d[tt * 128:(tt + 1) * 128, :],
                                      in_=yo)

                psos = {}
                for tt in range(2):
                    psos[tt] = psA.tile([128, 1024], F32, tag="pp", bufs=2,
                                        name=f"pso{tt}")
                    yproj_partial(psos[tt], tt)
                for tt in range(8):
                    if tt not in psos:
                        psos[tt] = psA.tile([128, 1024], F32, tag="pp",
                                            bufs=2, name=f"pso{tt}")
                        yproj_partial(psos[tt], tt)
                    yproj_finish(psos.pop(tt), tt)
                for tt in range(8):
                    psm = psA.tile([128, 1024], F16, tag="pp", bufs=2,
                                   name=f"psm{tt}")
                    for te in range(8):
                        nc.tensor.transpose(
                            psm[:, te * 128:(te + 1) * 128],
                            am_a[:, te, tt * 128:(tt + 1) * 128],
                            ident16,
                        )
                    mo = p2.tile([128, 1024], F16, tag="yo", bufs=3,
                                 name=f"mo{tt}")
                    # am_a already holds att/16 (exp bias fold) — plain copy
                    if tt % 2 == 0:
                        nc.scalar.copy(mo, psm)
                    else:
                        nc.vector.tensor_copy(mo, psm)
                    nc.sync.dma_start(out=am_d[tt * 128:(tt + 1) * 128, :],
                                      in_=mo)

    nc.finalize()
    return nc


def _get_nc(vp_bias=False):
    if vp_bias not in _NC:
        _NC[vp_bias] = _build(vp_bias=vp_bias)
    return _NC[vp_bias]


def _numpy_fallback(x, enc, mask, wq, bq, wk, bk, wv, bv, wp, bp):
    B, Tt, Cc = x.shape
    q = (x @ wq + bq).reshape(B, Tt, H, DH)
    k = (enc @ wk + bk).reshape(B, enc.shape[1], H, DH)
    v = (enc @ wv + bv).reshape(B, enc.shape[1], H, DH)
    att = np.einsum("bqhd,bkhd->bhqk", q, k).astype(np.float32) * SCALE
    att = np.where(mask[:, None, :, :], -np.inf, att)
    att = att - att.max(axis=-1, keepdims=True)
    att = np.exp(att)
    att = att / att.sum(axis=-1, keepdims=True)
    y = np.einsum("bhqk,bkhd->bqhd", att, v).reshape(B, Tt, Cc)
    am = att.mean(axis=1)
    y = y @ wp + bp
    return y.astype(np.float32), am.astype(np.float32)


def _sbuf_major(a):
    """[R, cols] -> [128, R//128, cols] with row index = a*128+p."""
    r, cols = a.shape
    return np.ascontiguousarray(a.reshape(r // 128, 128, cols).transpose(1, 0, 2))


def _run(inputs, trace=False):
    x = np.asarray(inputs["x"], dtype=np.float32)
    enc = np.asarray(inputs["encoder_output"], dtype=np.float32)
    mask = np.asarray(inputs["mask"])
    wq = np.asarray(inputs["Wq"], dtype=np.float32)
    wk = np.asarray(inputs["Wk"], dtype=np.float32)
    wv = np.asarray(inputs["Wv"], dtype=np.float32)
    wp = np.asarray(inputs["Wp"], dtype=np.float32)
    bq = np.asarray(inputs["bq"], dtype=np.float32).reshape(1, C)
    bk = np.asarray(inputs["bk"], dtype=np.float32).reshape(1, C)
    bv = np.asarray(inputs["bv"], dtype=np.float32).reshape(1, C)
    bp = np.asarray(inputs["bp"], dtype=np.float32).reshape(1, C)

    if mask.any():
        return _numpy_fallback(x, enc, mask,
                               wq, bq[0], wk, bk[0], wv, bv[0],
                               wp, bp[0]), None

    # Wp is pre-scaled x16: the device computes y/16 (exp bias -ln16 folds
    # the head-mean /16 into attx), so the proj weight absorbs the x16.
    w16 = {n: _sbuf_major(w.astype(np.float16))
           for n, w in (("wv", wv), ("wp", wp * np.float32(16.0)))}
    # wq/wk chunked per co-chunk / head-pair: [8, 128, 8, 128]
    wq16 = np.ascontiguousarray(
        _sbuf_major(wq.astype(np.float16))
        .reshape(128, 8, 8, 128).transpose(2, 0, 1, 3))
    wk16 = np.ascontiguousarray(
        _sbuf_major(wk.astype(np.float16))
        .reshape(128, 8, 8, 128).transpose(2, 0, 1, 3))
    bq_sb = np.ascontiguousarray(bq.reshape(8, 128).T.astype(np.float32))
    bk_sb = np.ascontiguousarray(bk.reshape(8, 128).T.astype(np.float32))
    x16 = x.astype(np.float16)
    enc16 = enc.astype(np.float16)

    nc = _get_nc(vp_bias=bool(bv.any() or bp.any()))
    in_maps = []
    encT_cache = {}
    for c in range(8):
        b, th = divmod(c, 2)
        if b not in encT_cache:
            encT_cache[b] = _sbuf_major(np.ascontiguousarray(enc16[b].T))
        in_maps.append({
            "xh": _sbuf_major(np.ascontiguousarray(
                x16[b, th * T:(th + 1) * T].T)),
            "enc": encT_cache[b],
            "wq": wq16, "wk": wk16,
            "wv": w16["wv"], "wp": w16["wp"],
            "bq": bq_sb, "bk": bk_sb,
            "bv": bv.astype(np.float16), "bp": bp.astype(np.float16),
        })
    res = run_bass_kernel_spmd(nc, in_maps, core_ids=list(range(8)),
                               trace=trace)
    B = x.shape[0]
    y = np.empty((B, 2 * T, C), np.float32)
    am = np.empty((B, 2 * T, TE), np.float32)
    for c in range(8):
        b, th = divmod(c, 2)
        y[b, th * T:(th + 1) * T] = res.results[c]["y"].astype(np.float32)
        am[b, th * T:(th + 1) * T] = res.results[c]["am"].astype(np.float32)
    return (y, am), res


def kernel(**inputs):
    out, _ = _run(inputs, trace=False)
    return out

